# revision 7
# baseline (speedup 1.0000x reference)
"""BSplineWarp Trainium2 kernel.

The reference computes:
  up     = bicubic_resize(displacements, 1024, 1024)        # [N, 2, H, W]
  deltas = grid_pull_cubic(up, identity_grid)               # cubic B-spline sample
  out    = image_coordinates + moveaxis(deltas, 1, -1)

Because the sampling grid is the integer identity grid, the fractional part of
every sample coordinate is 0, so the cubic B-spline weights collapse to the
constant 3-tap stencil [1/6, 4/6, 1/6] per axis (replicate border).  Both the
bicubic upsample and that smoothing are separable linear maps along each image
axis, so the whole displacement field is exactly

  deltas[n, c] = M @ D[n, c] @ M^T,   M = S_smooth @ B_bicubic   # [1024, 32]

with M a constant [1024, 32] matrix precomputed on the host.  The device
kernel is then two tiny matmul stages per tile plus a memory-bound streaming
add with image_coordinates.

Sharding: data-parallel over the transforms axis — core i handles n in
[2i, 2i+2).  No cross-core communication.
"""

import numpy as np

N_FULL = 16
N_CORES = 8
N_PER = N_FULL // N_CORES  # transforms per core
H = W = 1024
HC = 32  # coarse control grid
RCHUNKS = H // 128  # row chunks of 128 per image

_A = -0.75  # torch bicubic coefficient


def _cubic_conv_w(t):
    offs = np.arange(-1.0, 3.0)
    d = np.abs(t[None, :] - offs[:, None])
    w_near = ((_A + 2.0) * d - (_A + 3.0)) * d * d + 1.0
    w_far = _A * (((d - 5.0) * d + 8.0) * d - 4.0)
    return np.where(d <= 1.0, w_near, np.where(d < 2.0, w_far, 0.0))


def _upsample_matrix(in_size, out_size):
    # Row o of B holds the bicubic taps: resize_last(x) == x @ B.T
    B = np.zeros((out_size, in_size))
    scale = in_size / out_size
    pos = (np.arange(out_size) + 0.5) * scale - 0.5
    i0 = np.floor(pos)
    t = pos - i0
    idx = np.clip(i0.astype(np.int64)[None, :] + np.arange(-1, 3)[:, None], 0, in_size - 1)
    w = _cubic_conv_w(t)
    for k in range(4):
        for o in range(out_size):
            B[o, idx[k, o]] += w[k, o]
    return B


def _smooth_matrix(n):
    # Cubic B-spline at integer sample points: [1/6, 4/6, 1/6], replicate clamp
    S = np.zeros((n, n))
    w = (1.0 / 6.0, 4.0 / 6.0, 1.0 / 6.0)
    for o in range(n):
        for d in (-1, 0, 1):
            S[o, min(max(o + d, 0), n - 1)] += w[d + 1]
    return S


def _host_matrices():
    M = (_smooth_matrix(H) @ _upsample_matrix(HC, H)).astype(np.float32)  # [1024, 32]
    Mt = np.ascontiguousarray(M.T)  # [32, 1024]
    # Channel-interleaved variant: out columns are (x, c) pairs so the second
    # matmul writes deltas already in the [..., x, c] memory order of the output.
    Mint = np.zeros((2 * HC, 2 * W), np.float32)  # [64, 2048]
    Mint[:HC, 0::2] = Mt
    Mint[HC:, 1::2] = Mt
    return Mt, Mint


_MODULE_CACHE = {}


def _build_module(reps=1):
    # reps>1 repeats the whole body (same work, same I/O) for wall-clock
    # benchmarking by differencing; the graded path uses reps=1.
    import concourse.bacc as bacc
    import concourse.mybir as mybir
    from concourse.tile import TileContext

    f32 = mybir.dt.float32
    Mt, Mint = _host_matrices()

    nc = bacc.Bacc("TRN2", debug=False, num_devices=N_CORES)

    coords = nc.dram_tensor("coords", [N_PER, H, W, 2], f32, kind="ExternalInput")
    disp = nc.dram_tensor("disp", [N_PER, 2, HC, HC], f32, kind="ExternalInput")
    out = nc.dram_tensor("out", [N_PER, H, W, 2], f32, kind="ExternalOutput")
    mt_d = nc.inline_tensor(Mt, "mt_const")
    mint_d = nc.inline_tensor(Mint, "mint_const")

    coords_r = coords.ap().rearrange("n (ry p) w c -> n ry p (w c)", p=128)
    out_r = out.ap().rearrange("n (ry p) w c -> n ry p (w c)", p=128)
    disp_ap = disp.ap()

    with TileContext(nc) as tc:
        with (
            tc.tile_pool(name="const", bufs=1) as cpool,
            tc.tile_pool(name="tt", bufs=2) as ttpool,
            tc.tile_pool(name="io", bufs=4) as iopool,
            tc.tile_pool(name="ptt", bufs=2, space="PSUM") as pttpool,
            tc.tile_pool(name="pd", bufs=3, space="PSUM") as pdpool,
        ):
            mt_sb = cpool.tile([HC, H], f32)
            nc.sync.dma_start(out=mt_sb[:], in_=mt_d.ap())
            mint_sb = cpool.tile([2 * HC, 2 * W], f32)
            nc.sync.dma_start(out=mint_sb[:], in_=mint_d.ap())
            disp_sb = cpool.tile([HC, N_PER * 2 * HC], f32)
            for n in range(N_PER):
                for c in range(2):
                    s = (n * 2 + c) * HC
                    nc.sync.dma_start(out=disp_sb[:, s : s + HC], in_=disp_ap[n, c])

            def body(n, r):
                ct = iopool.tile([128, 2 * W], f32, tag="io", name="ct")
                nc.sync.dma_start(out=ct[:], in_=coords_r[n, r])

                # Tt_c = (M_r @ D_c)^T = D_c^T @ M_r^T   [32 coarse-x, 128 y]
                ptt0 = pttpool.tile([HC, 128], f32, tag="ptt", name="ptt0")
                ptt1 = pttpool.tile([HC, 128], f32, tag="ptt", name="ptt1")
                rsl = mt_sb[:, r * 128 : (r + 1) * 128]
                s0 = (n * 2 + 0) * HC
                s1 = (n * 2 + 1) * HC
                nc.tensor.matmul(
                    ptt0[:], disp_sb[:, s0 : s0 + HC], rsl, start=True, stop=True
                )
                nc.tensor.matmul(
                    ptt1[:], disp_sb[:, s1 : s1 + HC], rsl, start=True, stop=True
                )

                # stack both channels: TT [64, 128]
                tt = ttpool.tile([2 * HC, 128], f32, tag="tt", name="tt")
                nc.scalar.copy(out=tt[:HC], in_=ptt0[:])
                nc.scalar.copy(out=tt[HC:], in_=ptt1[:])

                # deltas chunk, channel-interleaved: [128 y, 2048 (x,c)]
                pd0 = pdpool.tile([128, 1024], f32, tag="pd", name="pd0")
                pd1 = pdpool.tile([128, 1024], f32, tag="pd", name="pd1")
                for q in range(2):
                    nc.tensor.matmul(
                        pd0[:, q * 512 : (q + 1) * 512],
                        tt[:],
                        mint_sb[:, q * 512 : (q + 1) * 512],
                        start=True,
                        stop=True,
                    )
                    nc.tensor.matmul(
                        pd1[:, q * 512 : (q + 1) * 512],
                        tt[:],
                        mint_sb[:, 1024 + q * 512 : 1024 + (q + 1) * 512],
                        start=True,
                        stop=True,
                    )

                nc.vector.tensor_add(out=ct[:, :1024], in0=ct[:, :1024], in1=pd0[:])
                nc.vector.tensor_add(out=ct[:, 1024:], in0=ct[:, 1024:], in1=pd1[:])

                nc.sync.dma_start(out=out_r[n, r], in_=ct[:])

            for _rep in range(reps):
                for n in range(N_PER):
                    for r in range(RCHUNKS):
                        body(n, r)

    nc.compile()
    return nc


def _get_module(reps=1):
    if reps not in _MODULE_CACHE:
        _MODULE_CACHE[reps] = _build_module(reps)
    return _MODULE_CACHE[reps]


def _run(inputs, trace=False, reps=1, **spmd_kwargs):
    from concourse import bass_utils

    nc = _get_module(reps)
    coords = np.ascontiguousarray(inputs["image_coordinates"], dtype=np.float32)
    disp = np.ascontiguousarray(inputs["displacements"], dtype=np.float32)
    in_maps = [
        {
            "coords": coords[i * N_PER : (i + 1) * N_PER],
            "disp": disp[i * N_PER : (i + 1) * N_PER],
        }
        for i in range(N_CORES)
    ]
    res = bass_utils.run_bass_kernel_spmd(
        nc, in_maps, core_ids=list(range(N_CORES)), trace=trace, **spmd_kwargs
    )
    full = np.concatenate([res.results[i]["out"] for i in range(N_CORES)], axis=0)
    return full, res


def kernel(image_coordinates, displacements):
    full, _ = _run(
        {"image_coordinates": image_coordinates, "displacements": displacements}
    )
    return full


# revision 10
# speedup vs baseline: 95.2621x; 95.2621x over previous
"""BSplineWarp Trainium2 kernel.

The reference computes:
  up     = bicubic_resize(displacements, 1024, 1024)        # [N, 2, H, W]
  deltas = grid_pull_cubic(up, identity_grid)               # cubic B-spline sample
  out    = image_coordinates + moveaxis(deltas, 1, -1)

Because the sampling grid is the integer identity grid, the fractional part of
every sample coordinate is 0, so the cubic B-spline weights collapse to the
constant 3-tap stencil [1/6, 4/6, 1/6] per axis (replicate border).  Both the
bicubic upsample and that smoothing are separable linear maps along each image
axis, so the whole displacement field is exactly

  deltas[n, c] = M @ D[n, c] @ M^T,   M = S_smooth @ B_bicubic   # [1024, 32]

with M a constant [1024, 32] matrix precomputed on the host.  The device
kernel is then two tiny matmul stages per tile plus a memory-bound streaming
add with image_coordinates.

Sharding: data-parallel over the transforms axis — core i handles n in
[2i, 2i+2).  No cross-core communication.
"""

import numpy as np

N_FULL = 16
N_CORES = 8
N_PER = N_FULL // N_CORES  # transforms per core
H = W = 1024
HC = 32  # coarse control grid
RCHUNKS = H // 128  # row chunks of 128 per image

_A = -0.75  # torch bicubic coefficient


def _cubic_conv_w(t):
    offs = np.arange(-1.0, 3.0)
    d = np.abs(t[None, :] - offs[:, None])
    w_near = ((_A + 2.0) * d - (_A + 3.0)) * d * d + 1.0
    w_far = _A * (((d - 5.0) * d + 8.0) * d - 4.0)
    return np.where(d <= 1.0, w_near, np.where(d < 2.0, w_far, 0.0))


def _upsample_matrix(in_size, out_size):
    # Row o of B holds the bicubic taps: resize_last(x) == x @ B.T
    B = np.zeros((out_size, in_size))
    scale = in_size / out_size
    pos = (np.arange(out_size) + 0.5) * scale - 0.5
    i0 = np.floor(pos)
    t = pos - i0
    idx = np.clip(i0.astype(np.int64)[None, :] + np.arange(-1, 3)[:, None], 0, in_size - 1)
    w = _cubic_conv_w(t)
    for k in range(4):
        for o in range(out_size):
            B[o, idx[k, o]] += w[k, o]
    return B


def _smooth_matrix(n):
    # Cubic B-spline at integer sample points: [1/6, 4/6, 1/6], replicate clamp
    S = np.zeros((n, n))
    w = (1.0 / 6.0, 4.0 / 6.0, 1.0 / 6.0)
    for o in range(n):
        for d in (-1, 0, 1):
            S[o, min(max(o + d, 0), n - 1)] += w[d + 1]
    return S


def _host_matrices():
    M = (_smooth_matrix(H) @ _upsample_matrix(HC, H)).astype(np.float32)  # [1024, 32]
    Mt = np.ascontiguousarray(M.T)  # [32, 1024]
    # Channel-interleaved variant: out columns are (x, c) pairs so the second
    # matmul writes deltas already in the [..., x, c] memory order of the output.
    Mint = np.zeros((2 * HC, 2 * W), np.float32)  # [64, 2048]
    Mint[:HC, 0::2] = Mt
    Mint[HC:, 1::2] = Mt
    return Mt, Mint


_MODULE_CACHE = {}


def _build_module(reps=1, dyn_reps=1):
    # reps>1 (python unroll) or dyn_reps>1 (hardware For_i loop) repeat the
    # whole body (same work, same I/O) for wall-clock benchmarking by
    # differencing; the graded path uses reps=1, dyn_reps=1.
    import concourse.bacc as bacc
    import concourse.mybir as mybir
    from concourse.tile import TileContext

    f32 = mybir.dt.float32
    Mt, Mint = _host_matrices()

    nc = bacc.Bacc("TRN2", debug=False, num_devices=N_CORES)

    coords = nc.dram_tensor("coords", [N_PER, H, W, 2], f32, kind="ExternalInput")
    disp = nc.dram_tensor("disp", [N_PER, 2, HC, HC], f32, kind="ExternalInput")
    out = nc.dram_tensor("out", [N_PER, H, W, 2], f32, kind="ExternalOutput")
    mt_d = nc.inline_tensor(Mt, "mt_const")
    mint_d = nc.inline_tensor(Mint, "mint_const")

    coords_r = coords.ap().rearrange("n (ry p) w c -> n ry p (w c)", p=128)
    out_r = out.ap().rearrange("n (ry p) w c -> n ry p (w c)", p=128)
    disp_ap = disp.ap()

    with TileContext(nc) as tc:
        with (
            tc.tile_pool(name="const", bufs=1) as cpool,
            tc.tile_pool(name="tt", bufs=2) as ttpool,
            tc.tile_pool(name="io", bufs=4) as iopool,
            tc.tile_pool(name="ptt", bufs=2, space="PSUM") as pttpool,
            tc.tile_pool(name="pd", bufs=3, space="PSUM") as pdpool,
        ):
            mt_sb = cpool.tile([HC, H], f32)
            nc.sync.dma_start(out=mt_sb[:], in_=mt_d.ap())
            mint_sb = cpool.tile([2 * HC, 2 * W], f32)
            nc.sync.dma_start(out=mint_sb[:], in_=mint_d.ap())
            disp_sb = cpool.tile([HC, N_PER * 2 * HC], f32)
            for n in range(N_PER):
                for c in range(2):
                    s = (n * 2 + c) * HC
                    nc.sync.dma_start(out=disp_sb[:, s : s + HC], in_=disp_ap[n, c])

            def body(n, r):
                ct = iopool.tile([128, 2 * W], f32, tag="io", name="ct")
                nc.sync.dma_start(out=ct[:], in_=coords_r[n, r])

                # Tt_c = (M_r @ D_c)^T = D_c^T @ M_r^T   [32 coarse-x, 128 y]
                ptt0 = pttpool.tile([HC, 128], f32, tag="ptt", name="ptt0")
                ptt1 = pttpool.tile([HC, 128], f32, tag="ptt", name="ptt1")
                rsl = mt_sb[:, r * 128 : (r + 1) * 128]
                s0 = (n * 2 + 0) * HC
                s1 = (n * 2 + 1) * HC
                nc.tensor.matmul(
                    ptt0[:], disp_sb[:, s0 : s0 + HC], rsl, start=True, stop=True
                )
                nc.tensor.matmul(
                    ptt1[:], disp_sb[:, s1 : s1 + HC], rsl, start=True, stop=True
                )

                # stack both channels: TT [64, 128]
                tt = ttpool.tile([2 * HC, 128], f32, tag="tt", name="tt")
                nc.scalar.copy(out=tt[:HC], in_=ptt0[:])
                nc.scalar.copy(out=tt[HC:], in_=ptt1[:])

                # deltas chunk, channel-interleaved: [128 y, 2048 (x,c)]
                pd0 = pdpool.tile([128, 1024], f32, tag="pd", name="pd0")
                pd1 = pdpool.tile([128, 1024], f32, tag="pd", name="pd1")
                for q in range(2):
                    nc.tensor.matmul(
                        pd0[:, q * 512 : (q + 1) * 512],
                        tt[:],
                        mint_sb[:, q * 512 : (q + 1) * 512],
                        start=True,
                        stop=True,
                    )
                    nc.tensor.matmul(
                        pd1[:, q * 512 : (q + 1) * 512],
                        tt[:],
                        mint_sb[:, 1024 + q * 512 : 1024 + (q + 1) * 512],
                        start=True,
                        stop=True,
                    )

                nc.vector.tensor_add(out=ct[:, :1024], in0=ct[:, :1024], in1=pd0[:])
                nc.vector.tensor_add(out=ct[:, 1024:], in0=ct[:, 1024:], in1=pd1[:])

                nc.sync.dma_start(out=out_r[n, r], in_=ct[:])

            def one_rep():
                for n in range(N_PER):
                    for r in range(RCHUNKS):
                        body(n, r)

            if dyn_reps > 1:
                with tc.For_i(0, dyn_reps, 1):
                    one_rep()
            else:
                for _rep in range(reps):
                    one_rep()

    nc.compile()
    return nc


def _get_module(reps=1, dyn_reps=1):
    key = (reps, dyn_reps)
    if key not in _MODULE_CACHE:
        _MODULE_CACHE[key] = _build_module(reps, dyn_reps)
    return _MODULE_CACHE[key]


def _run(inputs, trace=False, reps=1, dyn_reps=1, **spmd_kwargs):
    from concourse import bass_utils

    nc = _get_module(reps, dyn_reps)
    coords = np.ascontiguousarray(inputs["image_coordinates"], dtype=np.float32)
    disp = np.ascontiguousarray(inputs["displacements"], dtype=np.float32)
    in_maps = [
        {
            "coords": coords[i * N_PER : (i + 1) * N_PER],
            "disp": disp[i * N_PER : (i + 1) * N_PER],
        }
        for i in range(N_CORES)
    ]
    res = bass_utils.run_bass_kernel_spmd(
        nc, in_maps, core_ids=list(range(N_CORES)), trace=trace, **spmd_kwargs
    )
    full = np.concatenate([res.results[i]["out"] for i in range(N_CORES)], axis=0)
    return full, res


def kernel(image_coordinates, displacements):
    full, _ = _run(
        {"image_coordinates": image_coordinates, "displacements": displacements}
    )
    return full


# revision 11
# speedup vs baseline: 103.7368x; 1.0890x over previous
"""BSplineWarp Trainium2 kernel.

The reference computes:
  up     = bicubic_resize(displacements, 1024, 1024)        # [N, 2, H, W]
  deltas = grid_pull_cubic(up, identity_grid)               # cubic B-spline sample
  out    = image_coordinates + moveaxis(deltas, 1, -1)

Because the sampling grid is the integer identity grid, the fractional part of
every sample coordinate is 0, so the cubic B-spline weights collapse to the
constant 3-tap stencil [1/6, 4/6, 1/6] per axis (replicate border).  Both the
bicubic upsample and that smoothing are separable linear maps along each image
axis, so the whole displacement field is exactly

  deltas[n, c] = M @ D[n, c] @ M^T,   M = S_smooth @ B_bicubic   # [1024, 32]

with M a constant [1024, 32] matrix precomputed on the host.  The device
kernel is then two tiny matmul stages per tile plus a memory-bound streaming
add with image_coordinates.

Sharding: data-parallel over the transforms axis — core i handles n in
[2i, 2i+2).  No cross-core communication.
"""

import numpy as np

N_FULL = 16
N_CORES = 8
N_PER = N_FULL // N_CORES  # transforms per core
H = W = 1024
HC = 32  # coarse control grid
RCHUNKS = H // 128  # row chunks of 128 per image

_A = -0.75  # torch bicubic coefficient


def _cubic_conv_w(t):
    offs = np.arange(-1.0, 3.0)
    d = np.abs(t[None, :] - offs[:, None])
    w_near = ((_A + 2.0) * d - (_A + 3.0)) * d * d + 1.0
    w_far = _A * (((d - 5.0) * d + 8.0) * d - 4.0)
    return np.where(d <= 1.0, w_near, np.where(d < 2.0, w_far, 0.0))


def _upsample_matrix(in_size, out_size):
    # Row o of B holds the bicubic taps: resize_last(x) == x @ B.T
    B = np.zeros((out_size, in_size))
    scale = in_size / out_size
    pos = (np.arange(out_size) + 0.5) * scale - 0.5
    i0 = np.floor(pos)
    t = pos - i0
    idx = np.clip(i0.astype(np.int64)[None, :] + np.arange(-1, 3)[:, None], 0, in_size - 1)
    w = _cubic_conv_w(t)
    for k in range(4):
        for o in range(out_size):
            B[o, idx[k, o]] += w[k, o]
    return B


def _smooth_matrix(n):
    # Cubic B-spline at integer sample points: [1/6, 4/6, 1/6], replicate clamp
    S = np.zeros((n, n))
    w = (1.0 / 6.0, 4.0 / 6.0, 1.0 / 6.0)
    for o in range(n):
        for d in (-1, 0, 1):
            S[o, min(max(o + d, 0), n - 1)] += w[d + 1]
    return S


def _host_matrices():
    M = (_smooth_matrix(H) @ _upsample_matrix(HC, H)).astype(np.float32)  # [1024, 32]
    Mt = np.ascontiguousarray(M.T)  # [32, 1024]
    # Channel-interleaved variant: out columns are (x, c) pairs so the second
    # matmul writes deltas already in the [..., x, c] memory order of the output.
    Mint = np.zeros((2 * HC, 2 * W), np.float32)  # [64, 2048]
    Mint[:HC, 0::2] = Mt
    Mint[HC:, 1::2] = Mt
    return Mt, Mint


_MODULE_CACHE = {}


def _build_module(reps=1, dyn_reps=1):
    # reps>1 (python unroll) or dyn_reps>1 (hardware For_i loop) repeat the
    # whole body (same work, same I/O) for wall-clock benchmarking by
    # differencing; the graded path uses reps=1, dyn_reps=1.
    import concourse.bacc as bacc
    import concourse.mybir as mybir
    from concourse.tile import TileContext

    f32 = mybir.dt.float32
    Mt, Mint = _host_matrices()

    nc = bacc.Bacc("TRN2", debug=False, num_devices=N_CORES)

    coords = nc.dram_tensor("coords", [N_PER, H, W, 2], f32, kind="ExternalInput")
    disp = nc.dram_tensor("disp", [N_PER, 2, HC, HC], f32, kind="ExternalInput")
    out = nc.dram_tensor("out", [N_PER, H, W, 2], f32, kind="ExternalOutput")
    mt_d = nc.inline_tensor(Mt, "mt_const")
    mint_d = nc.inline_tensor(Mint, "mint_const")

    coords_r = coords.ap().rearrange("n (ry p) w c -> n ry p (w c)", p=128)
    out_r = out.ap().rearrange("n (ry p) w c -> n ry p (w c)", p=128)
    disp_ap = disp.ap()

    with TileContext(nc) as tc:
        with (
            tc.tile_pool(name="const", bufs=1) as cpool,
            tc.tile_pool(name="tt", bufs=2) as ttpool,
            tc.tile_pool(name="io", bufs=4) as iopool,
            tc.tile_pool(name="ptt", bufs=1, space="PSUM") as pttpool,
            tc.tile_pool(name="pd", bufs=3, space="PSUM") as pdpool,
        ):
            mt_sb = cpool.tile([HC, H], f32)
            nc.sync.dma_start(out=mt_sb[:], in_=mt_d.ap())
            mint_sb = cpool.tile([2 * HC, 2 * W], f32)
            nc.sync.dma_start(out=mint_sb[:], in_=mint_d.ap())
            # disp as [ky partitions, (n c kx)] so lhsT slices are direct
            disp_sb = cpool.tile([HC, N_PER * 2 * HC], f32)
            for n in range(N_PER):
                for c in range(2):
                    s = (n * 2 + c) * HC
                    nc.sync.dma_start(out=disp_sb[:, s : s + HC], in_=disp_ap[n, c])

            def body(n):
                # TT = (M @ D)^T for all rows at once: [64 (c,kx), 1024 y].
                # One matmul pair + one PSUM->SBUF copy per transform.
                ptt_all = pttpool.tile([2 * HC, W], f32, tag="ptt", name="ptt_all")
                s = n * 2 * HC
                for q in range(2):
                    nc.tensor.matmul(
                        ptt_all[:, q * 512 : (q + 1) * 512],
                        disp_sb[:, s : s + 2 * HC],
                        mt_sb[:, q * 512 : (q + 1) * 512],
                        start=True,
                        stop=True,
                    )
                tt_all = ttpool.tile([2 * HC, W], f32, tag="tt", name="tt_all")
                nc.scalar.copy(out=tt_all[:], in_=ptt_all[:])

                for r in range(RCHUNKS):
                    ct = iopool.tile([128, 2 * W], f32, tag="io", name="ct")
                    nc.sync.dma_start(out=ct[:], in_=coords_r[n, r])

                    # deltas chunk, channel-interleaved: [128 y, 2048 (x,c)]
                    pd0 = pdpool.tile([128, 1024], f32, tag="pd", name="pd0")
                    pd1 = pdpool.tile([128, 1024], f32, tag="pd", name="pd1")
                    lhs = tt_all[:, r * 128 : (r + 1) * 128]
                    for q in range(2):
                        nc.tensor.matmul(
                            pd0[:, q * 512 : (q + 1) * 512],
                            lhs,
                            mint_sb[:, q * 512 : (q + 1) * 512],
                            start=True,
                            stop=True,
                        )
                        nc.tensor.matmul(
                            pd1[:, q * 512 : (q + 1) * 512],
                            lhs,
                            mint_sb[:, 1024 + q * 512 : 1024 + (q + 1) * 512],
                            start=True,
                            stop=True,
                        )

                    nc.vector.tensor_add(out=ct[:, :1024], in0=ct[:, :1024], in1=pd0[:])
                    nc.vector.tensor_add(out=ct[:, 1024:], in0=ct[:, 1024:], in1=pd1[:])

                    # store on the ACT HWDGE ring so its sem waits never block
                    # load issuance on the SP ring
                    nc.scalar.dma_start(out=out_r[n, r], in_=ct[:])

            def one_rep():
                for n in range(N_PER):
                    body(n)

            if dyn_reps > 1:
                with tc.For_i(0, dyn_reps, 1):
                    one_rep()
            else:
                for _rep in range(reps):
                    one_rep()

    nc.compile()
    return nc


def _get_module(reps=1, dyn_reps=1):
    key = (reps, dyn_reps)
    if key not in _MODULE_CACHE:
        _MODULE_CACHE[key] = _build_module(reps, dyn_reps)
    return _MODULE_CACHE[key]


def _run(inputs, trace=False, reps=1, dyn_reps=1, **spmd_kwargs):
    from concourse import bass_utils

    nc = _get_module(reps, dyn_reps)
    coords = np.ascontiguousarray(inputs["image_coordinates"], dtype=np.float32)
    disp = np.ascontiguousarray(inputs["displacements"], dtype=np.float32)
    in_maps = [
        {
            "coords": coords[i * N_PER : (i + 1) * N_PER],
            "disp": disp[i * N_PER : (i + 1) * N_PER],
        }
        for i in range(N_CORES)
    ]
    res = bass_utils.run_bass_kernel_spmd(
        nc, in_maps, core_ids=list(range(N_CORES)), trace=trace, **spmd_kwargs
    )
    full = np.concatenate([res.results[i]["out"] for i in range(N_CORES)], axis=0)
    return full, res


def kernel(image_coordinates, displacements):
    full, _ = _run(
        {"image_coordinates": image_coordinates, "displacements": displacements}
    )
    return full
